# revision 1
# baseline (speedup 1.0000x reference)
"""Trainium2 Bass kernel for nn_Block_27212912788319 (dense transformer block).

Sharding: 8 NeuronCores = 2 batch groups (cores 0-3 -> batch 0, cores 4-7 ->
batch 1) x 4-way tensor parallel. TP rank r holds 4 attention heads (heads
sorted by ALiBi slope and dealt round-robin so per-slot causal pruning stays
tight across the shared SPMD program), 1/4 of the p (gated-MLP) features, the
matching rows of W_in and columns of W_out. Mid-layernorm statistics are
all-reduced (tiny) inside each TP group; out_proj partials are reduce-scattered
so each rank runs the final layernorm for a quarter of the tokens.

All module compute runs on device. The host only slices / transposes / casts
weights, builds small ALiBi lookup tables from the slopes, and reassembles the
output.

Layouts on device (per core):
  - x arrives token-major [L, HID]; input LN is done token-major (free-axis
    stats), then PE-transposed to feature-major xs^T [HID, L] with the
    channel shifts applied as column offsets during the transpose copy-out.
  - in_proj / attention / pg / out_proj run feature-major.
  - attention computes S^T = K Q^T chunks (j on partitions); ALiBi is added
    exactly via a host-built f32 rank-1 table plus a per-chunk constant in the
    Exp activation bias; softmax normalization happens at the end via a
    ones-column appended to V (column sums ride along the AV matmul).
  - out_proj produces token-major partials; ReduceScatter over the TP group
    hands each rank its token quarter, which it layernorms and outputs.
"""

import sys
from contextlib import ExitStack

for _p in ("/opt/trn_rl_repo", "/root/.axon_site/_ro/trn_rl_repo"):
    if _p not in sys.path:
        sys.path.insert(0, _p)

import numpy as np
import ml_dtypes

import concourse.bass as bass
import concourse.bacc as bacc
import concourse.mybir as mybir
from concourse import tile
from concourse.bass_utils import run_bass_kernel_spmd

F32 = mybir.dt.float32
BF16 = mybir.dt.bfloat16
AF = mybir.ActivationFunctionType
ALU = mybir.AluOpType
AX = mybir.AxisListType

# ---------------- problem constants ----------------
B, H, D = 2, 16, 64
HID = H * D                  # 1024
EF = 4
QKVP = HID * (3 + EF)        # 7168
PFULL = HID * EF             # 4096
TP = 4                       # tensor-parallel ranks per batch group
HPC = H // TP                # 4 heads ("slots") per core
QW = HPC * D                 # 256
PW = PFULL // TP             # 1024
LOCF = 3 * QW + PW           # 1792 local in_proj features
EPS = 1e-5
C_SAFE = 32.0                # softmax range shift (cancels exactly)
PRUNE_MARGIN = 92.0          # drop attention chunks with weight < e^-92
NEG = -1e30

BLK = 512                    # token block
JC = 128                     # j-chunk / token-chunk
NKC = HID // 128             # 8 contraction chunks for in_proj
NMC = LOCF // 128            # 14 full M chunks
OKC = (QW + PW) // 128       # 10 out_proj contraction chunks
NCORES = 8
LAST_RESULT = None
REPLICA_GROUPS = [[0, 1, 2, 3], [4, 5, 6, 7]]


def _prune_dists(slopes_sorted_desc, L):
    """Per-slot keep distance from the MIN slope across the slot's 4 ranks
    (one shared program => pruning is the union over ranks)."""
    d = []
    for s in range(HPC):
        smin = float(min(slopes_sorted_desc[TP * s: TP * s + TP]))
        if smin <= PRUNE_MARGIN / (L + JC):
            d.append(L + JC)
        else:
            d.append(int(np.ceil(PRUNE_MARGIN / smin)))
    return d


def _kept_chunks(dist, blk):
    i0 = blk * BLK
    njc = (i0 + BLK) // JC
    jc_min = max(0, -(-(i0 - (JC - 1) - dist) // JC))
    return list(range(jc_min, njc))


def build_program(L, prune_dists, identity_outln,
                  identity_inln=True, identity_midln=True):
    NBLK = L // BLK
    TOKC = L // 128
    NJ = L // JC
    nc = bacc.Bacc(None, target_bir_lowering=False)

    # ---------------- I/O ----------------
    x_in = nc.dram_tensor("x", [L, HID], F32, kind="ExternalInput")
    w_inT = nc.dram_tensor("w_inT", [HID, LOCF + 1], BF16, kind="ExternalInput")
    w_outT = nc.dram_tensor("w_outT", [QW + PW, HID], BF16, kind="ExternalInput")
    mid_g = nc.dram_tensor("mid_g", [LOCF + 1], F32, kind="ExternalInput")
    mid_b = nc.dram_tensor("mid_b", [LOCF + 1], F32, kind="ExternalInput")
    in_g = nc.dram_tensor("in_g", [HID], F32, kind="ExternalInput")
    in_b = nc.dram_tensor("in_b", [HID], F32, kind="ExternalInput")
    out_g = nc.dram_tensor("out_g", [HID], F32, kind="ExternalInput")
    out_b = nc.dram_tensor("out_b", [HID], F32, kind="ExternalInput")
    t_alibi = nc.dram_tensor("t_alibi", [HPC, JC, BLK], F32, kind="ExternalInput")
    tri_d = nc.dram_tensor("tri", [JC, JC], F32, kind="ExternalInput")
    eyeb_d = nc.dram_tensor("eyeb", [2 * D, D], BF16, kind="ExternalInput")
    eyef_d = nc.dram_tensor("eyef", [128, 128], F32, kind="ExternalInput")
    ccb_d = nc.dram_tensor("ccb", [HPC, 128, 4 * NBLK], F32, kind="ExternalInput")
    y_out = nc.dram_tensor("y", [L // TP, HID], F32, kind="ExternalOutput")

    # internal DRAM for collectives
    st_in = nc.dram_tensor("st_in", [NBLK, 2, BLK], F32)
    st_out = nc.dram_tensor("st_out", [NBLK, 2, BLK], F32)
    po_in = nc.dram_tensor("po_in", [NBLK, BLK, HID], F32)
    po_out = nc.dram_tensor("po_out", [NBLK, BLK // TP, HID], F32)

    ctx = ExitStack()
    with ctx:
        tc = ctx.enter_context(tile.TileContext(nc))

        # ---------------- persistent tiles ----------------
        pers = ctx.enter_context(tc.tile_pool(name="pers", bufs=1))
        xs = [pers.tile([128, L + 4], BF16, name=f"xs{c}") for c in range(NKC)]
        hqkv = [pers.tile([128, L], BF16, name=f"hqkv{c}") for c in range(6)]
        vhat = [pers.tile([128, NJ * (D + 1)], BF16, name=f"vhat{s}")
                for s in range(HPC)]
        wi = [pers.tile([128, LOCF + 1], BF16, name=f"wi{c}") for c in range(NKC)]
        wo = [pers.tile([128, HID], BF16, name=f"wo{c}") for c in range(OKC)]
        tal = [pers.tile([128, BLK], F32, name=f"tal{s}") for s in range(HPC)]
        ccbs = [pers.tile([128, 4 * NBLK], F32, name=f"ccb{s}")
                for s in range(HPC)]
        tri_t = pers.tile([128, JC], F32, name="tri_t")
        eyeb_t = pers.tile([2 * D, D], BF16, name="eyeb_t")
        eyef_t = pers.tile([128, 128], F32, name="eyef_t")
        ones_t = pers.tile([128, 1], BF16, name="ones_t")
        eps_t = pers.tile([128, 1], F32, name="eps_t")
        onesf_t = pers.tile([128, 128], F32, name="onesf_t")
        midg_t = pers.tile([128, NMC + 1], F32, name="midg_t")
        midb_t = pers.tile([128, NMC + 1], F32, name="midb_t")
        ing_t = pers.tile([128, NKC], F32, name="ing_t")
        inb_t = pers.tile([128, NKC], F32, name="inb_t")
        if not identity_outln:
            og_row = pers.tile([1, HID], F32, name="og_row")
            ob_row = pers.tile([1, HID], F32, name="ob_row")
            og_bc = pers.tile([128, HID], F32, name="og_bc")
            ob_bc = pers.tile([128, HID], F32, name="ob_bc")

        for c in range(NKC):
            nc.sync.dma_start(wi[c][:, :], w_inT[128 * c:128 * (c + 1), :])
            nc.sync.dma_start(ing_t[:, c:c + 1], in_g[128 * c:128 * (c + 1)])
            nc.sync.dma_start(inb_t[:, c:c + 1], in_b[128 * c:128 * (c + 1)])
        for c in range(OKC):
            nc.sync.dma_start(wo[c][:, :], w_outT[128 * c:128 * (c + 1), :])
        for s in range(HPC):
            nc.sync.dma_start(tal[s][:, :], t_alibi[s])
            nc.sync.dma_start(ccbs[s][:, :], ccb_d[s])
        nc.sync.dma_start(tri_t[:, :], tri_d[:, :])
        nc.sync.dma_start(eyeb_t[:, :], eyeb_d[:, :])
        nc.sync.dma_start(eyef_t[:, :], eyef_d[:, :])
        for c in range(NMC):
            nc.sync.dma_start(midg_t[:, c:c + 1], mid_g[128 * c:128 * (c + 1)])
            nc.sync.dma_start(midb_t[:, c:c + 1], mid_b[128 * c:128 * (c + 1)])
        nc.sync.dma_start(midg_t[0:1, NMC:NMC + 1], mid_g[LOCF:LOCF + 1])
        nc.sync.dma_start(midb_t[0:1, NMC:NMC + 1], mid_b[LOCF:LOCF + 1])
        if not identity_outln:
            nc.sync.dma_start(og_row[0:1, :], out_g[:])
            nc.sync.dma_start(ob_row[0:1, :], out_b[:])
        nc.vector.memset(ones_t[:, :], 1.0)
        nc.vector.memset(eps_t[:, :], EPS)
        nc.vector.memset(onesf_t[:, :], 1.0)
        for s in range(HPC):
            nc.vector.memset(vhat[s][:, :], 1.0)
        nc.vector.memset(xs[0][:, 0:1], 0.0)
        nc.vector.memset(xs[1][:, 0:3], 0.0)

        # ---------------- PSUM pools ----------------
        pmm = ctx.enter_context(tc.tile_pool(name="pmm", bufs=3, space="PSUM"))
        pav_pool = ctx.enter_context(tc.tile_pool(name="pav", bufs=2,
                                                  space="PSUM"))
        pstat_pool = ctx.enter_context(tc.tile_pool(name="pstat", bufs=1,
                                                    space="PSUM"))
        ptr_pool = ctx.enter_context(tc.tile_pool(name="ptr", bufs=1,
                                                  space="PSUM"))

        if not identity_outln:
            for half in range(HID // BLK):
                sl = slice(BLK * half, BLK * (half + 1))
                pg1 = pmm.tile([128, BLK], F32, tag="mm")
                nc.tensor.matmul(pg1[:, :], onesf_t[0:1, :], og_row[0:1, sl],
                                 start=True, stop=True)
                nc.vector.tensor_copy(og_bc[:, sl], pg1[:, :])
                pg2 = pmm.tile([128, BLK], F32, tag="mm")
                nc.tensor.matmul(pg2[:, :], onesf_t[0:1, :], ob_row[0:1, sl],
                                 start=True, stop=True)
                nc.vector.tensor_copy(ob_bc[:, sl], pg2[:, :])

        # shared big f32 [128, HID] scratch (input LN + final LN)
        big = ctx.enter_context(tc.tile_pool(name="big", bufs=2))

        def ln_rows_col(pool_tile, n_feat):
            """Column variant: cols 0 (sum), 1 (sumsq) of a [128, 8] tile ->
            cols 2..7 = m, e2, var, std, r, -m*r."""
            t = pool_tile
            nc.vector.tensor_scalar_mul(t[:, 2:3], t[:, 0:1], 1.0 / n_feat)
            nc.vector.tensor_scalar_mul(t[:, 3:4], t[:, 1:2], 1.0 / n_feat)
            nc.vector.scalar_tensor_tensor(
                out=t[:, 4:5], in0=t[:, 2:3], scalar=-1.0,
                in1=t[:, 2:3], op0=ALU.mult, op1=ALU.mult)
            nc.vector.tensor_add(t[:, 4:5], t[:, 4:5], t[:, 3:4])
            nc.scalar.activation(t[:, 5:6], t[:, 4:5], AF.Sqrt,
                                 bias=eps_t[:, 0:1])
            nc.vector.reciprocal(t[:, 6:7], t[:, 5:6])
            nc.vector.scalar_tensor_tensor(
                out=t[:, 7:8], in0=t[:, 2:3], scalar=-1.0,
                in1=t[:, 6:7], op0=ALU.mult, op1=ALU.mult)

        # ---------------- input layernorm, transpose, shift ----------------
        SHIFT = {0: 1, 1: 3}
        for t in range(TOKC):
            xt = big.tile([128, HID], F32, tag="bx")
            nc.sync.dma_start(xt[:, :], x_in[128 * t:128 * (t + 1), :])
            srow = big.tile([128, 8], F32, tag="bsrow")
            sq = big.tile([128, HID], F32, tag="bscr", bufs=1)
            nc.vector.tensor_reduce(srow[:, 0:1], xt[:, :], axis=AX.X,
                                    op=ALU.add)
            nc.vector.tensor_mul(sq[:, :], xt[:, :], xt[:, :])
            nc.vector.tensor_reduce(srow[:, 1:2], sq[:, :], axis=AX.X,
                                    op=ALU.add)
            ln_rows_col(srow, HID)
            xn = big.tile([128, HID], F32, tag="bxn")
            nc.vector.tensor_scalar(xn[:, :], xt[:, :], srow[:, 6:7],
                                    srow[:, 7:8], ALU.mult, ALU.add)
            for c in range(NKC):
                pt = ptr_pool.tile([128, 128], F32, tag="ptr")
                nc.tensor.transpose(pt[:, :], xn[:, 128 * c:128 * (c + 1)],
                                    eyef_t[:, :])
                d0 = 128 * t + SHIFT.get(c, 0)
                if identity_inln:
                    nc.scalar.activation(xs[c][:, d0:d0 + 128], pt[:, :],
                                         AF.Identity)
                else:
                    nc.vector.tensor_scalar(xs[c][:, d0:d0 + 128], pt[:, :],
                                            ing_t[:, c:c + 1],
                                            inb_t[:, c:c + 1],
                                            ALU.mult, ALU.add)

        # ---------------- per-block pools ----------------
        bp = ctx.enter_context(tc.tile_pool(name="bp", bufs=2))
        catp = ctx.enter_context(tc.tile_pool(name="catp", bufs=1))

        for b in range(NBLK):
            i0 = b * BLK
            # ---------- in_proj + relu + stats ----------
            ps_sum = pstat_pool.tile([1, BLK], F32, tag="ssum")
            ps_sq = pstat_pool.tile([1, BLK], F32, tag="ssq")
            h_tiles = []
            for mc in range(NMC + 1):
                pm = pmm.tile([128, BLK], F32, tag="mm")
                mw = 128 if mc < NMC else 1
                for kc in range(NKC):
                    nc.tensor.matmul(
                        pm[0:mw, :],
                        wi[kc][:, 128 * mc:128 * mc + mw],
                        xs[kc][:, i0:i0 + BLK],
                        start=(kc == 0), stop=(kc == NKC - 1))
                ht = bp.tile([mw, BLK], BF16, tag=f"hh{mc}")
                nc.scalar.activation(ht[:, :], pm[0:mw, :], AF.Relu)
                h_tiles.append(ht)
                if mc < NMC:
                    h2 = bp.tile([128, BLK], BF16, tag="hsq", bufs=1)
                    nc.vector.tensor_mul(h2[:, :], ht[:, :], ht[:, :])
                    nc.tensor.matmul(ps_sum[0:1, :], ones_t[:, :], ht[:, :],
                                     start=(mc == 0), stop=(mc == NMC - 1))
                    nc.tensor.matmul(ps_sq[0:1, :], ones_t[:, :], h2[:, :],
                                     start=(mc == 0), stop=(mc == NMC - 1))
            # ---------- stats all-reduce ----------
            stats_sb = bp.tile([1, 2 * BLK], F32, tag="stats", bufs=1)
            nc.vector.tensor_copy(stats_sb[0:1, 0:BLK], ps_sum[0:1, :])
            nc.vector.tensor_copy(stats_sb[0:1, BLK:2 * BLK], ps_sq[0:1, :])
            nc.sync.dma_start(st_in[b][0:1, :], stats_sb[0:1, 0:BLK])
            nc.sync.dma_start(st_in[b][1:2, :], stats_sb[0:1, BLK:2 * BLK])
            nc.gpsimd.collective_compute(
                "AllReduce", ALU.add, replica_groups=REPLICA_GROUPS,
                ins=[st_in[b]], outs=[st_out[b]])
            rows = bp.tile([1, 2 * BLK], F32, tag="stats", bufs=1)
            nc.sync.dma_start(rows[0:1, 0:BLK], st_out[b][0:1, :])
            nc.sync.dma_start(rows[0:1, BLK:2 * BLK], st_out[b][1:2, :])
            mb = bp.tile([128, BLK], F32, tag="mb", bufs=1)
            rb = bp.tile([128, BLK], F32, tag="rb", bufs=1)
            lnscr = bp.tile([128, BLK], F32, tag="lnscr", bufs=1)
            pmb = pmm.tile([128, BLK], F32, tag="mm")
            nc.tensor.matmul(pmb[:, :], onesf_t[0:1, :], rows[0:1, 0:BLK],
                             start=True, stop=True)
            nc.vector.tensor_scalar_mul(mb[:, :], pmb[:, :], 1.0 / QKVP)
            prb = pmm.tile([128, BLK], F32, tag="mm")
            nc.tensor.matmul(prb[:, :], onesf_t[0:1, :],
                             rows[0:1, BLK:2 * BLK], start=True, stop=True)
            nc.vector.tensor_scalar_mul(rb[:, :], prb[:, :], 1.0 / QKVP)
            # rb: e2 -> var -> std -> r   (all base-0 [128, BLK] ops)
            nc.vector.scalar_tensor_tensor(
                out=lnscr[:, :], in0=mb[:, :], scalar=-1.0, in1=mb[:, :],
                op0=ALU.mult, op1=ALU.mult)
            nc.vector.tensor_add(rb[:, :], rb[:, :], lnscr[:, :])
            nc.scalar.activation(rb[:, :], rb[:, :], AF.Sqrt,
                                 bias=eps_t[:, 0:1])
            nc.vector.reciprocal(rb[:, :], rb[:, :])

            # ---------- mid-LN normalize (+ gelu / pg) ----------
            cat_tiles = [catp.tile([128, BLK], BF16, tag=f"c{k}", name=f"c{k}")
                         for k in range(OKC)]
            phat = []
            pbound = None
            # boundary feature (mc == NMC) first so pg chunk 0 can use it
            for mc in [NMC] + list(range(NMC)):
                mw = 128 if mc < NMC else 1
                t1 = bp.tile([mw, BLK], F32, tag="t1", bufs=1)
                nc.vector.tensor_sub(t1[:, :], h_tiles[mc][:, :], mb[0:mw, :])
                if mc < 6:
                    dest = hqkv[mc][:, i0:i0 + BLK]
                elif mc < NMC:
                    pt_ = bp.tile([128, BLK], BF16, tag="phat", bufs=3)
                    phat.append(pt_)
                    dest = pt_[:, :]
                else:
                    pbound = bp.tile([1, BLK], BF16, tag="pbound", bufs=1)
                    dest = pbound[:, :]
                if identity_midln:
                    nc.vector.tensor_mul(dest, t1[:, :], rb[0:mw, :])
                else:
                    nc.vector.tensor_mul(t1[:, :], t1[:, :], rb[0:mw, :])
                    nc.vector.tensor_scalar(dest, t1[:, :],
                                            midg_t[0:mw, mc:mc + 1],
                                            midb_t[0:mw, mc:mc + 1],
                                            ALU.mult, ALU.add)
                if 6 <= mc < NMC:
                    pc = mc - 6
                    gel = bp.tile([128, BLK], BF16, tag="gel", bufs=1)
                    nc.scalar.activation(gel[:, :], phat[pc][:, :], AF.Gelu)
                    psh = bp.tile([128, BLK], BF16, tag="psh", bufs=1)
                    nc.sync.dma_start(psh[1:128, :], phat[pc][0:127, :])
                    prev_row = (pbound[0:1, :] if pc == 0
                                else phat[pc - 1][127:128, :])
                    nc.sync.dma_start(psh[0:1, :], prev_row)
                    ct = cat_tiles[2 + pc]
                    nc.vector.tensor_mul(ct[:, :], gel[:, :], psh[:, :])

            # ---------- v transposes ----------
            for s in range(HPC):
                for u in range(BLK // JC):
                    jj = b * (BLK // JC) + u
                    vsrc = hqkv[4 + s // 2][64 * (s % 2):64 * (s % 2) + 64,
                                            JC * jj:JC * (jj + 1)]
                    pt = ptr_pool.tile([128, D], BF16, tag="ptr")
                    off = 64 * (s % 2)
                    nc.tensor.transpose(pt[:, :], vsrc,
                                        eyeb_t[off:off + D, :])
                    nc.vector.tensor_copy(
                        vhat[s][:, (D + 1) * jj:(D + 1) * jj + D], pt[:, :])

            # ---------- attention ----------
            for s in range(HPC):
                q_ap = hqkv[s // 2][64 * (s % 2):64 * (s % 2) + 64,
                                    i0:i0 + BLK]
                pav = pav_pool.tile([D + 1, BLK], F32, tag="av")
                kept = _kept_chunks(prune_dists[s], b)
                for idx, jc in enumerate(kept):
                    delta = JC * jc - i0
                    c0 = max(0, delta)
                    k_ap = hqkv[2 + s // 2][64 * (s % 2):64 * (s % 2) + 64,
                                            JC * jc:JC * (jc + 1)]
                    pS = pmm.tile([128, BLK], F32, tag="mm")
                    nc.tensor.matmul(pS[:, c0:BLK], k_ap, q_ap[:, c0:BLK],
                                     start=True, stop=True)
                    nc.vector.scalar_tensor_tensor(
                        out=pS[:, c0:BLK], in0=pS[:, c0:BLK], scalar=0.125,
                        in1=tal[s][:, c0:BLK], op0=ALU.mult, op1=ALU.add)
                    if delta >= 0:
                        nc.vector.tensor_add(pS[:, delta:delta + JC],
                                             pS[:, delta:delta + JC],
                                             tri_t[:, :])
                    pp = bp.tile([128, BLK], BF16, tag="pp")
                    col = jc - (BLK // JC) * b + (BLK // JC) * (NBLK - 1)
                    nc.scalar.activation(pp[:, c0:BLK], pS[:, c0:BLK], AF.Exp,
                                         bias=ccbs[s][:, col:col + 1])
                    nc.tensor.matmul(
                        pav[:, c0:BLK],
                        vhat[s][:, (D + 1) * jc:(D + 1) * (jc + 1)],
                        pp[:, c0:BLK],
                        start=(idx == 0), stop=(idx == len(kept) - 1))
                zrow = bp.tile([128, BLK], F32, tag="lnscr", bufs=1)
                nc.vector.reciprocal(zrow[64:65, :], pav[D:D + 1, :])
                zb = bp.tile([D, BLK], F32, tag="zb", bufs=1)
                pzb = pmm.tile([128, BLK], F32, tag="mm")
                nc.tensor.matmul(pzb[0:D, :], onesf_t[64:65, 0:D],
                                 zrow[64:65, :], start=True, stop=True)
                nc.vector.tensor_copy(zb[:, :], pzb[0:D, :])
                nc.vector.tensor_mul(
                    cat_tiles[s // 2][64 * (s % 2):64 * (s % 2) + 64, :],
                    pav[0:D, :], zb[:, :])

            # ---------- out_proj ----------
            for tcn in range(BLK // 128):
                for nn in range(HID // BLK):
                    po = pmm.tile([128, BLK], F32, tag="mm")
                    for kc in range(OKC):
                        nc.tensor.matmul(
                            po[:, :],
                            cat_tiles[kc][:, 128 * tcn:128 * (tcn + 1)],
                            wo[kc][:, BLK * nn:BLK * (nn + 1)],
                            start=(kc == 0), stop=(kc == OKC - 1))
                    pos = bp.tile([128, BLK], F32, tag="pos", bufs=1)
                    nc.scalar.copy(pos[:, :], po[:, :])
                    nc.sync.dma_start(
                        po_in[b][128 * tcn:128 * (tcn + 1),
                                 BLK * nn:BLK * (nn + 1)], pos[:, :])

            # ---------- reduce-scatter + final LN ----------
            nc.gpsimd.collective_compute(
                "ReduceScatter", ALU.add, replica_groups=REPLICA_GROUPS,
                ins=[po_in[b]], outs=[po_out[b]])
            ft = big.tile([128, HID], F32, tag="bx")
            nc.sync.dma_start(ft[:, :], po_out[b])
            frow = big.tile([128, 8], F32, tag="bsrow")
            fsq = big.tile([128, HID], F32, tag="bscr", bufs=1)
            nc.vector.tensor_reduce(frow[:, 0:1], ft[:, :], axis=AX.X,
                                    op=ALU.add)
            nc.vector.tensor_mul(fsq[:, :], ft[:, :], ft[:, :])
            nc.vector.tensor_reduce(frow[:, 1:2], fsq[:, :], axis=AX.X,
                                    op=ALU.add)
            ln_rows_col(frow, HID)
            fn = big.tile([128, HID], F32, tag="bxn")
            nc.vector.tensor_scalar(fn[:, :], ft[:, :], frow[:, 6:7],
                                    frow[:, 7:8], ALU.mult, ALU.add)
            if not identity_outln:
                nc.vector.tensor_mul(fn[:, :], fn[:, :], og_bc[:, :])
                nc.vector.tensor_add(fn[:, :], fn[:, :], ob_bc[:, :])
            nc.sync.dma_start(y_out[128 * b:128 * (b + 1), :], fn[:, :])

    return nc


# ---------------- host side ----------------

def _bf16(a):
    return np.asarray(a, dtype=np.float32).astype(ml_dtypes.bfloat16)


def prep_inputs(x, in_ln_g, in_ln_b, W_in, mid_ln_g, mid_ln_b, slopes,
                W_out, out_ln_g, out_ln_b, L):
    """Build the 8 per-core input maps + the build-time config."""
    NBLK = L // BLK
    slopes = np.asarray(slopes, dtype=np.float32)
    order = np.argsort(-slopes, kind="stable")      # descending
    sorted_slopes = slopes[order]
    prune = _prune_dists(sorted_slopes, L)
    identity_outln = (np.allclose(out_ln_g, 1.0) and np.allclose(out_ln_b, 0.0))
    identity_inln = (np.allclose(in_ln_g, 1.0) and np.allclose(in_ln_b, 0.0))
    identity_midln = (np.allclose(mid_ln_g, 1.0)
                      and np.allclose(mid_ln_b, 0.0))

    tri = np.where(np.arange(JC)[:, None] > np.arange(JC)[None, :],
                   np.float32(NEG), np.float32(0.0)).astype(np.float32)
    eyeb = np.vstack([np.eye(D, dtype=np.float32)] * 2).astype(ml_dtypes.bfloat16)
    eyef = np.eye(128, dtype=np.float32)

    in_maps = []
    meta = []
    for core in range(NCORES):
        bb, r = core // TP, core % TP
        heads = [int(order[TP * s + r]) for s in range(HPC)]
        rows = []
        for part in range(3):
            for h in heads:
                rows += list(range(part * HID + h * D, part * HID + (h + 1) * D))
        p0 = r * PW
        rows += list(range(3 * HID + p0, 3 * HID + p0 + PW))
        rows.append(3 * HID + (p0 - 1) % PFULL)
        rows = np.asarray(rows)
        cols = []
        for h in heads:
            cols += list(range(h * D, (h + 1) * D))
        cols += list(range(HID + p0, HID + p0 + PW))
        cols = np.asarray(cols)

        jr = np.arange(JC, dtype=np.float32)[:, None]
        ic = np.arange(BLK, dtype=np.float32)[None, :]
        t_alibi = np.stack([slopes[h] * (jr - ic) for h in heads]
                           ).astype(np.float32)
        ncols = 4 * NBLK
        ccb = np.zeros((HPC, 128, ncols), np.float32)
        for s, h in enumerate(heads):
            for col in range(ncols):
                delta = 128.0 * (col - (BLK // JC) * (NBLK - 1))
                ccb[s, :, col] = slopes[h] * delta - C_SAFE

        in_maps.append({
            "x": np.ascontiguousarray(x[bb], dtype=np.float32),
            "w_inT": np.ascontiguousarray(_bf16(W_in[rows]).T),
            "w_outT": np.ascontiguousarray(_bf16(W_out[:, cols]).T),
            "mid_g": np.ascontiguousarray(mid_ln_g[rows]).astype(np.float32),
            "mid_b": np.ascontiguousarray(mid_ln_b[rows]).astype(np.float32),
            "in_g": np.asarray(in_ln_g, dtype=np.float32),
            "in_b": np.asarray(in_ln_b, dtype=np.float32),
            "out_g": np.asarray(out_ln_g, dtype=np.float32),
            "out_b": np.asarray(out_ln_b, dtype=np.float32),
            "t_alibi": t_alibi,
            "tri": tri,
            "eyeb": eyeb,
            "eyef": eyef,
            "ccb": ccb,
        })
        meta.append((bb, r))
    return in_maps, meta, prune, (identity_outln, identity_inln, identity_midln)


def unshard(results, meta, L):
    NBLK = L // BLK
    out = np.zeros((B, L, HID), np.float32)
    for core, (bb, r) in enumerate(meta):
        y = results[core]["y"]          # [L//TP, HID]
        for blk in range(NBLK):
            out[bb, BLK * blk + 128 * r: BLK * blk + 128 * r + 128, :] = \
                y[128 * blk:128 * (blk + 1), :]
    return out


def kernel(**inputs):
    L = inputs["x"].shape[1]
    in_maps, meta, prune, ident = prep_inputs(
        inputs["x"], inputs["in_ln_g"], inputs["in_ln_b"], inputs["W_in"],
        inputs["mid_ln_g"], inputs["mid_ln_b"], inputs["slopes"],
        inputs["W_out"], inputs["out_ln_g"], inputs["out_ln_b"], L)
    nc = build_program(L, prune, ident[0], ident[1], ident[2])
    nc.finalize()
    res = run_bass_kernel_spmd(nc, in_maps, list(range(NCORES)))
    global LAST_RESULT
    LAST_RESULT = res
    return unshard(res.results, meta, L)


if __name__ == "__main__":
    print("kernel module; use test.py")

